# revision 25
# baseline (speedup 1.0000x reference)
"""Tied-row (MSA) attention, sharded over 8 TRN2 NeuronCores.

Reference computation (b=1, r=128 MSA rows, n=512, 8 heads x 64):
    q, k, v = x @ Wq, x @ Wk, x @ Wv          per-row projections
    dots[h,i,j] = sum_{r,d} q[r,h,i,d] k[r,h,j,d] * scale / sqrt(num_rows)
    attn = softmax_j(dots)                     shared across rows
    out[r,i] = (sum_j attn[h,i,j] v[r,h,j,d]) @ Wo + bo

Sharding: MSA-row axis r split 16-per-core.  Each core computes its partial
logits (reduction over its local r); partials are summed with one bf16
AllReduce per head-pair, pipelined behind the following pairs' matmuls.

This revision computes the logits TRANSPOSED (dotsT[j,i] via stationary=kT),
so the softmax needs no PE transposes: exp runs directly on the AllReduced
dotsT tiles, the denominator Z[i] comes from a ones-vector matmul over the
partition (j) axis, 1/Z is broadcast across partitions with a rank-1
fp32r matmul, and the normalization is an elementwise bf16 multiply.
attn^T @ v and the output projection are core-local.  x is cast to bf16
before its PE transposes (bf16 transposes run 2x faster than fp32);
the attn@v outputs and Wo are bf16 so the output-projection weight loads
hit the fast (FWL) path.
"""

import numpy as np

import concourse.bacc as bacc
import concourse.bass as bass
import concourse.mybir as mybir
import concourse.tile as tile
from concourse import bass_utils
from concourse.masks import make_identity

CORES = 8
R = 16          # MSA rows per core
N = 512         # sequence length
DIM = 256       # model dim
H = 8           # heads
D = 64          # head dim
HD = H * D      # 512
RN = R * N      # 8192 token-rows per core

F32 = mybir.dt.float32
F32R = mybir.dt.float32r
BF16 = mybir.dt.bfloat16

RG = [list(range(CORES))]


def build_nc(scale: float):
    nc = bacc.Bacc(None, target_bir_lowering=False, debug=False)

    x_ext = nc.declare_dram_parameter("x", [RN, DIM], F32, isOutput=False)
    wq_ext = nc.declare_dram_parameter("wq", [DIM, HD], F32, isOutput=False)
    wk_ext = nc.declare_dram_parameter("wk", [DIM, HD], F32, isOutput=False)
    wv_ext = nc.declare_dram_parameter("wv", [DIM, HD], F32, isOutput=False)
    wo_ext = nc.declare_dram_parameter("wo", [HD, DIM], F32, isOutput=False)
    out_ext = nc.declare_dram_parameter("out", [RN, DIM], F32, isOutput=True)

    # alternate PSUM->SBUF copies between DVE and ScalarE so neither gates
    # PSUM-bank recycling
    _cp = [0]

    def cp(out, in_):
        if _cp[0] % 2 == 0:
            nc.vector.tensor_copy(out, in_)
        else:
            nc.scalar.copy(out, in_)
        _cp[0] += 1

    # all bulk DMAs ride the Sync queue: the GpSimd queue must stay
    # collectives-only (instructions behind a collective_compute wait for
    # the collective to finish), and DMAs on Scalar/Vector would head-block
    # the PSUM evictions those engines perform
    def dma(out, in_):
        nc.sync.dma_start(out=out, in_=in_)

    with tile.TileContext(nc) as tc:
        # ---- DRAM bounce buffers: one AllReduce per head-pair ----
        dram = tc.alloc_tile_pool(name="dram", bufs=1, space="DRAM")
        ar_in = [dram.tile([2 * N, N], BF16, tag=f"ar_in{hp}", name=f"ar_in{hp}") for hp in range(4)]
        wu_in = dram.tile([128, 8], BF16, tag="wu_in", name="wu_in")
        wu_out = dram.tile([128, 8], BF16, tag="wu_out", name="wu_out", addr_space="Shared")
        ar_out = [
            dram.tile([2 * N, N], BF16, tag=f"ar_out{hp}", name=f"ar_out{hp}", addr_space="Shared")
            for hp in range(4)
        ]

        # ---- pools (allocated up front; releases must be LIFO per space) ----
        consts = tc.alloc_tile_pool(name="consts", bufs=1)
        v_pool = tc.alloc_tile_pool(name="v", bufs=R * 4)
        attnT_pool = tc.alloc_tile_pool(name="attnT", bufs=1)
        xT_pool = tc.alloc_tile_pool(name="xT", bufs=1)
        xrow_pool = tc.alloc_tile_pool(name="xrow", bufs=8)
        xbf_pool = tc.alloc_tile_pool(name="xbf", bufs=8)
        wstage = tc.alloc_tile_pool(name="wstage", bufs=2)

        # first x rows prefetch, ahead of everything else on the sync queue
        first_xrs = []
        for c in range(4):
            xr = xrow_pool.tile([128, DIM], F32, tag="xr")
            nc.sync.dma_start(out=xr[:], in_=x_ext[c * 128:(c + 1) * 128, :])
            first_xrs.append(xr)

        # ---- constants (weight DMAs are emitted after the x loads; the x
        # stream paces the whole front of the kernel) ----
        wq_sb = consts.tile([128, 2, HD], BF16, tag="wq")
        wk_sb = consts.tile([128, 2, HD], BF16, tag="wk")
        wv_sb = consts.tile([128, 2, HD], BF16, tag="wv")
        wo_r = consts.tile([128, 4, DIM], F32R, tag="wor")
        idbf = consts.tile([128, 128], BF16, tag="idbf")
        ones_f = consts.tile([128, 128], F32, tag="ones_f")
        ones_bf = consts.tile([128, 128], BF16, tag="ones_bf")
        make_identity(nc, idbf[:])
        nc.vector.memset(ones_f[:], 1.0)
        nc.vector.tensor_copy(ones_bf[:], ones_f[:])

        # warm up ncfw so the first real AllReduce skips the cold-start lag
        nc.sync.dma_start(out=wu_in[:, :], in_=idbf[:, 0:8])
        nc.gpsimd.collective_compute(
            "AllReduce",
            mybir.AluOpType.add,
            replica_groups=RG,
            ins=[wu_in[:, :].opt()],
            outs=[wu_out[:, :].opt()],
        )

        attn = attnT_pool.tile([128, H, 4, N], BF16, tag="attn")
        xT = xT_pool.tile([128, 2, RN], BF16, tag="xT")

        proj_psum = tc.alloc_tile_pool(name="proj_psum", bufs=3, space="PSUM")
        dots_psum = tc.alloc_tile_pool(name="dots_psum", bufs=3, space="PSUM")
        xp_psum = tc.alloc_tile_pool(name="xp_psum", bufs=2, space="PSUM")

        # ---- load x, cast to bf16, transpose to x^T [dim(2x128), rn] ----
        # 4 PE transposes batched per PSUM bank -> one [128,512] copy out
        _cast = [0]
        for c4 in range(RN // N):
            xbs = []
            for j in range(4):
                c = c4 * 4 + j
                if c < 4:
                    xr = first_xrs[c]
                else:
                    xr = xrow_pool.tile([128, DIM], F32, tag="xr")
                    # the x load paces the whole kernel front and is
                    # queue-serialized; put 3 of every 8 chunks on the Scalar
                    # queue (idle early) so two DMAs run concurrently
                    if c % 8 >= 5:
                        nc.scalar.dma_start(out=xr[:], in_=x_ext[c * 128:(c + 1) * 128, :])
                    else:
                        nc.sync.dma_start(out=xr[:], in_=x_ext[c * 128:(c + 1) * 128, :])
                xb = xbf_pool.tile([128, DIM], BF16, tag="xb")
                # fp32->bf16 casts: round-robin DVE / ScalarE (GpSimd would
                # sit behind the warmup collective on its queue)
                if _cast[0] % 2 == 0:
                    nc.vector.tensor_copy(xb[:], xr[:])
                else:
                    nc.scalar.copy(xb[:], xr[:])
                _cast[0] += 1
                xbs.append(xb)
            for kc in range(2):
                pt = xp_psum.tile([128, N], BF16, tag="xp")
                for j in range(4):
                    nc.tensor.transpose(
                        pt[:, j * 128:(j + 1) * 128],
                        xbs[j][:, kc * 128:(kc + 1) * 128],
                        idbf[:],
                    )
                cp(xT[:, kc, c4 * N:(c4 + 1) * N], pt[:])

        # weight staging, behind the x stream on the sync queue
        for wext, wsb in ((wq_ext, wq_sb), (wk_ext, wk_sb), (wv_ext, wv_sb)):
            wf = wstage.tile([128, 2, HD], F32, tag="wf")
            nc.sync.dma_start(
                out=wf[:], in_=wext[:, :].rearrange("(k p) n -> p k n", p=128)
            )
            nc.any.tensor_copy(wsb[:], wf[:])
        wof = wstage.tile([128, 4, DIM], F32, tag="wf")
        nc.sync.dma_start(
            out=wof[:], in_=wo_ext[:, :].rearrange("(k p) n -> p k n", p=128)
        )
        nc.any.tensor_copy(wo_r[:], wof[:])

        wstage.release()
        xbf_pool.release()
        xrow_pool.release()
        xp_psum.release()

        dstage_pool = tc.alloc_tile_pool(name="dstage", bufs=4)
        smax_pool = tc.alloc_tile_pool(name="smax", bufs=2)
        qkT_pool = tc.alloc_tile_pool(name="qkT", bufs=1)

        def softmax_local(hp, spool, zpool, sfx, wait_ms):
            """exp + transpose-free normalize of both heads of AllReduce #hp
            into attn[:, 2hp+m, :, :] (tiles are [j-part, i-free]).

            wait_ms biases the Tile scheduler: this whole chain is gated on
            AllReduce #hp, so model it as not-ready before then — otherwise
            its ops head-block the Sync/Scalar/Vector queues ahead of
            independent work (the scheduler's collective timing is a guess)."""
            with tc.tile_wait_until(wait_ms):
                for m in range(2):
                    h = 2 * hp + m
                    exps = []
                    for jc in range(4):
                        zt = spool.tile([128, N], BF16, tag="zt" + sfx)
                        row0 = m * N + jc * 128
                        dma(zt[:], ar_out[hp][row0:row0 + 128, :])
                        et = spool.tile([128, N], BF16, tag="et" + sfx, bufs=5)
                        nc.scalar.activation(
                            et[:], zt[:], mybir.ActivationFunctionType.Exp, scale=scale
                        )
                        exps.append(et)
                    # Z[i] broadcast to all partitions: all-ones stationary
                    # sums exp over the partition (j) axis into every row
                    bps = zpool.tile([128, N], F32, tag="bps" + sfx)
                    for jc in range(4):
                        nc.tensor.matmul(
                            bps[:],
                            ones_bf[:],
                            exps[jc][:],
                            start=(jc == 0),
                            stop=(jc == 3),
                        )
                    rz = spool.tile([128, N], BF16, tag="rz" + sfx)
                    with nc.allow_low_precision(reason="1/Z scale fine in bf16"):
                        nc.vector.reciprocal(rz[:], bps[:])
                    for jc in range(4):
                        nc.vector.tensor_mul(attn[:, h, jc, :], exps[jc][:], rz[:])

        for hp in range(4):
            qT = qkT_pool.tile([128, RN], BF16, tag="qT")
            kT = qkT_pool.tile([128, RN], BF16, tag="kT")
            for wsb, dstT in ((wq_sb, qT), (wk_sb, kT)):
                for ch in range(RN // N):
                    ps = proj_psum.tile([128, N], F32, tag="proj")
                    for kc in range(2):
                        nc.tensor.matmul(
                            ps[:],
                            wsb[:, kc, hp * 128:(hp + 1) * 128],
                            xT[:, kc, ch * N:(ch + 1) * N],
                            start=(kc == 0),
                            stop=(kc == 1),
                        )
                    cp(dstT[:, ch * N:(ch + 1) * N], ps[:])

            # partial dotsT[j,i] for the two heads of this pair; the even head
            # uses PE row-group 0-63, the odd head 64-127 (concurrent tiles)
            for jc in range(4):
                pe_ = dots_psum.tile([128, N], F32, tag="dots")
                po_ = dots_psum.tile([128, N], F32, tag="dots")
                for rr in range(R):
                    base = rr * N
                    jsl = slice(base + jc * 128, base + jc * 128 + 128)
                    isl = slice(base, base + N)
                    nc.tensor.matmul(
                        pe_[:],
                        kT[0:64, jsl],
                        qT[0:64, isl],
                        start=(rr == 0),
                        stop=(rr == R - 1),
                        skip_group_check=True,
                    )
                    nc.tensor.matmul(
                        po_[:],
                        kT[64:128, jsl],
                        qT[64:128, isl],
                        start=(rr == 0),
                        stop=(rr == R - 1),
                        skip_group_check=True,
                    )
                for m, ps in ((0, pe_), (1, po_)):
                    st = dstage_pool.tile([128, N], BF16, tag="dstage")
                    cp(st[:], ps[:])
                    row0 = m * N + jc * 128
                    dma(ar_in[hp][row0:row0 + 128, :], st[:])

            nc.gpsimd.collective_compute(
                "AllReduce",
                mybir.AluOpType.add,
                replica_groups=RG,
                ins=[ar_in[hp][:, :].opt()],
                outs=[ar_out[hp][:, :].opt()],
            )

        qkT_pool.release()
        z_psum = tc.alloc_tile_pool(name="z_psum", bufs=2, space="PSUM")

        # ---- v projection (overlaps the AllReduces; reads xT) ----
        v_tiles = {}
        for rr in range(R):
            if rr == 3:
                softmax_local(0, smax_pool, z_psum, "", 0.16)
            if rr == 9:
                softmax_local(1, smax_pool, z_psum, "", 0.21)
            for jt in range(4):
                ps = proj_psum.tile([128, N], F32, tag="proj")
                for kc in range(2):
                    nc.tensor.matmul(
                        ps[:],
                        xT[:, kc, rr * N + jt * 128:rr * N + jt * 128 + 128],
                        wv_sb[:, kc, :],
                        start=(kc == 0),
                        stop=(kc == 1),
                    )
                vt = v_pool.tile([128, HD], BF16, tag="v")
                cp(vt[:], ps[:])
                v_tiles[(rr, jt)] = vt

        z_psum.release()
        smax_pool.release()
        dstage_pool.release()
        xT_pool.release()
        dots_psum.release()
        proj_psum.release()

        # ---- attn^T @ v -> out^T (f32r), out @ Wo in three AR-gated phases:
        # phase A (pairs 0+1): attn@v for all rows + partial out-projection
        #   (Wo rows of heads 0-3) held in SBUF as bf16;
        # phase B (pair 2, after AllReduce 2): attn@v + accumulate its
        #   projection term into the partials;
        # phase C (pair 3, after AllReduce 3): attn@v + final term + store.
        # Post-last-AllReduce work shrinks from the full output projection to
        # one quarter of it.
        oT_pool = tc.alloc_tile_pool(name="oT", bufs=6)
        pf_pool = tc.alloc_tile_pool(name="pf", bufs=35)
        fstage_pool = tc.alloc_tile_pool(name="fstage", bufs=6)
        av_psum = tc.alloc_tile_pool(name="av_psum", bufs=3, space="PSUM")
        fin_psum = tc.alloc_tile_pool(name="fin_psum", bufs=3, space="PSUM")
        z2_psum = tc.alloc_tile_pool(name="z2_psum", bufs=2, space="PSUM")

        def attnv(rr, hp):
            ps = av_psum.tile([128, N], F32, tag="av")
            for jt in range(4):
                for m in range(2):
                    h = 2 * hp + m
                    nc.tensor.matmul(
                        ps[m * 64:(m + 1) * 64, :],
                        v_tiles[(rr, jt)][:, h * D:(h + 1) * D],
                        attn[:, h, jt, :],
                        start=(jt == 0),
                        stop=(jt == 3),
                        tile_position=(0, m * 64),
                        skip_group_check=True,
                    )
            oT = oT_pool.tile([128, N], F32R, tag="oT")
            cp(oT[:], ps[:])
            return oT

        _oq = [0]
        pf = {}
        for rr in range(R):
            o0 = attnv(rr, 0)
            o1 = attnv(rr, 1)
            for i2 in range(2):
                psA = fin_psum.tile([128, 2, DIM], F32, tag="fin")
                for xic in range(2):
                    ic = i2 * 2 + xic
                    nc.tensor.matmul(
                        psA[:, xic, :],
                        o0[:, ic * 128:(ic + 1) * 128],
                        wo_r[:, 0, :],
                        start=True,
                        stop=False,
                        skip_group_check=True,
                    )
                    nc.tensor.matmul(
                        psA[:, xic, :],
                        o1[:, ic * 128:(ic + 1) * 128],
                        wo_r[:, 1, :],
                        start=False,
                        stop=True,
                        skip_group_check=True,
                    )
                p = pf_pool.tile([128, 2, DIM], BF16, tag="pf")
                cp(p[:], psA[:])
                pf[(rr, i2)] = p

        softmax_local(2, fstage_pool, z2_psum, "2", 0.26)
        for rr in range(R):
            o2 = attnv(rr, 2)
            for i2 in range(2):
                psB = fin_psum.tile([128, 2, DIM], F32, tag="fin")
                for xic in range(2):
                    ic = i2 * 2 + xic
                    nc.tensor.matmul(
                        psB[:, xic, :],
                        o2[:, ic * 128:(ic + 1) * 128],
                        wo_r[:, 2, :],
                        start=True,
                        stop=True,
                        skip_group_check=True,
                    )
                p2 = pf_pool.tile([128, 2, DIM], BF16, tag="pf")
                nc.vector.tensor_add(p2[:], psB[:], pf[(rr, i2)][:])
                pf[(rr, i2)] = p2

        softmax_local(3, fstage_pool, z2_psum, "2", 0.30)
        for rr in range(R):
            o3 = attnv(rr, 3)
            for i2 in range(2):
                psC = fin_psum.tile([128, 2, DIM], F32, tag="fin")
                for xic in range(2):
                    ic = i2 * 2 + xic
                    nc.tensor.matmul(
                        psC[:, xic, :],
                        o3[:, ic * 128:(ic + 1) * 128],
                        wo_r[:, 3, :],
                        start=True,
                        stop=True,
                        skip_group_check=True,
                    )
                fst = fstage_pool.tile([128, 2, DIM], F32, tag="fst")
                nc.vector.tensor_add(fst[:], psC[:], pf[(rr, i2)][:])
                row0 = rr * N + i2 * 256
                dst = out_ext[row0:row0 + 256, :].rearrange("(k p) n -> p k n", p=128)
                # the whole 8 MB output leaves in this phase: use two queues
                # (the GpSimd queue is safely past all collectives here)
                if _oq[0] % 2 == 0:
                    nc.sync.dma_start(out=dst, in_=fst[:])
                else:
                    nc.gpsimd.dma_start(out=dst, in_=fst[:])
                _oq[0] += 1

        z2_psum.release()
        fin_psum.release()
        av_psum.release()
        fstage_pool.release()
        pf_pool.release()
        oT_pool.release()
        attnT_pool.release()
        v_pool.release()
        consts.release()
        dram.release()

    if not nc.is_finalized():
        nc.finalize()
    return nc


_cache = {}


def _get_nc(scale: float):
    key = round(float(scale), 12)
    if key not in _cache:
        _cache[key] = build_nc(float(scale))
    return _cache[key]


def make_in_maps(x, Wq, Wkv, Wo):
    x = np.ascontiguousarray(np.asarray(x, dtype=np.float32)).reshape(CORES, RN, DIM)
    Wq = np.ascontiguousarray(np.asarray(Wq, dtype=np.float32))
    Wkv = np.asarray(Wkv, dtype=np.float32)
    Wk = np.ascontiguousarray(Wkv[:, :HD])
    Wv = np.ascontiguousarray(Wkv[:, HD:])
    Wo = np.ascontiguousarray(np.asarray(Wo, dtype=np.float32))
    return [
        {"x": x[c], "wq": Wq, "wk": Wk, "wv": Wv, "wo": Wo} for c in range(CORES)
    ]


def kernel(x, Wq, Wkv, Wo, bo, mask, tie_attn_dim):
    x = np.asarray(x)
    br, n, dim = x.shape
    r = int(tie_attn_dim)
    assert (br, n, dim) == (128, 512, 256) and r == 128, "kernel hardcodes shapes"
    mask = np.asarray(mask)
    assert mask.all(), "kernel assumes an all-valid mask"
    num_rows = float(mask.reshape(1, r, n).any(axis=-1).sum(axis=-1)[0])
    scale = (D ** -0.5) * (num_rows ** -0.5)

    nc = _get_nc(scale)
    in_maps = make_in_maps(x, Wq, Wkv, Wo)
    res = bass_utils.run_bass_kernel_spmd(nc, in_maps, core_ids=list(range(CORES)))
    out = np.concatenate([m["out"] for m in res.results], axis=0)
    out = out.reshape(br, n, dim)
    bo = np.asarray(bo, dtype=np.float32)
    if bo.any():
        out = out + bo
    return np.ascontiguousarray(out.astype(np.float32))


# revision 29
# speedup vs baseline: 1.0854x; 1.0854x over previous
"""Tied-row (MSA) attention, sharded over 8 TRN2 NeuronCores.

Reference computation (b=1, r=128 MSA rows, n=512, 8 heads x 64):
    q, k, v = x @ Wq, x @ Wk, x @ Wv          per-row projections
    dots[h,i,j] = sum_{r,d} q[r,h,i,d] k[r,h,j,d] * scale / sqrt(num_rows)
    attn = softmax_j(dots)                     shared across rows
    out[r,i] = (sum_j attn[h,i,j] v[r,h,j,d]) @ Wo + bo

Sharding: MSA-row axis r split 16-per-core.  Each core computes its partial
logits (reduction over its local r); partials are summed with one bf16
AllReduce per head-pair, pipelined behind the following pairs' matmuls.

This revision computes the logits TRANSPOSED (dotsT[j,i] via stationary=kT),
so the softmax needs no PE transposes: exp runs directly on the AllReduced
dotsT tiles, the denominator Z[i] comes from a ones-vector matmul over the
partition (j) axis, 1/Z is broadcast across partitions with a rank-1
fp32r matmul, and the normalization is an elementwise bf16 multiply.
attn^T @ v and the output projection are core-local.  x is cast to bf16
before its PE transposes (bf16 transposes run 2x faster than fp32);
the attn@v outputs and Wo are bf16 so the output-projection weight loads
hit the fast (FWL) path.
"""

import numpy as np

import concourse.bacc as bacc
import concourse.bass as bass
import concourse.mybir as mybir
import concourse.tile as tile
from concourse import bass_utils
from concourse.masks import make_identity

CORES = 8
R = 16          # MSA rows per core
N = 512         # sequence length
DIM = 256       # model dim
H = 8           # heads
D = 64          # head dim
HD = H * D      # 512
RN = R * N      # 8192 token-rows per core

F32 = mybir.dt.float32
F32R = mybir.dt.float32r
BF16 = mybir.dt.bfloat16

RG = [list(range(CORES))]


def build_nc(scale: float):
    nc = bacc.Bacc(None, target_bir_lowering=False, debug=False)

    x_ext = nc.declare_dram_parameter("x", [RN, DIM], F32, isOutput=False)
    wq_ext = nc.declare_dram_parameter("wq", [DIM, HD], F32, isOutput=False)
    wk_ext = nc.declare_dram_parameter("wk", [DIM, HD], F32, isOutput=False)
    wv_ext = nc.declare_dram_parameter("wv", [DIM, HD], F32, isOutput=False)
    wo_ext = nc.declare_dram_parameter("wo", [HD, DIM], F32, isOutput=False)
    out_ext = nc.declare_dram_parameter("out", [RN, DIM], F32, isOutput=True)

    # alternate PSUM->SBUF copies between DVE and ScalarE so neither gates
    # PSUM-bank recycling
    _cp = [0]

    def cp(out, in_):
        if _cp[0] % 2 == 0:
            nc.vector.tensor_copy(out, in_)
        else:
            nc.scalar.copy(out, in_)
        _cp[0] += 1

    # all bulk DMAs ride the Sync queue: the GpSimd queue must stay
    # collectives-only (instructions behind a collective_compute wait for
    # the collective to finish), and DMAs on Scalar/Vector would head-block
    # the PSUM evictions those engines perform
    def dma(out, in_):
        nc.sync.dma_start(out=out, in_=in_)

    with tile.TileContext(nc) as tc:
        # ---- DRAM bounce buffers: one AllReduce per head-pair ----
        dram = tc.alloc_tile_pool(name="dram", bufs=1, space="DRAM")
        ar_in = [dram.tile([2 * N, N], BF16, tag=f"ar_in{hp}", name=f"ar_in{hp}") for hp in range(4)]
        wu_in = dram.tile([128, 8], BF16, tag="wu_in", name="wu_in")
        wu_out = dram.tile([128, 8], BF16, tag="wu_out", name="wu_out", addr_space="Shared")
        ar_out = [
            dram.tile([2 * N, N], BF16, tag=f"ar_out{hp}", name=f"ar_out{hp}", addr_space="Shared")
            for hp in range(4)
        ]

        # ---- pools (allocated up front; releases must be LIFO per space) ----
        consts = tc.alloc_tile_pool(name="consts", bufs=1)
        v_pool = tc.alloc_tile_pool(name="v", bufs=R * 4)
        attnT_pool = tc.alloc_tile_pool(name="attnT", bufs=1)
        xT_pool = tc.alloc_tile_pool(name="xT", bufs=1)
        xrow_pool = tc.alloc_tile_pool(name="xrow", bufs=8)
        xbf_pool = tc.alloc_tile_pool(name="xbf", bufs=8)
        wstage = tc.alloc_tile_pool(name="wstage", bufs=2)

        # first x rows prefetch, ahead of everything else on the sync queue
        first_xrs = []
        for c in range(4):
            xr = xrow_pool.tile([128, DIM], F32, tag="xr")
            nc.sync.dma_start(out=xr[:], in_=x_ext[c * 128:(c + 1) * 128, :])
            first_xrs.append(xr)

        # ---- constants; weight DMAs ride the Scalar queue so they neither
        # delay the x stream (sync queue) nor wait behind it ----
        wq_sb = consts.tile([128, 2, HD], BF16, tag="wq")
        wk_sb = consts.tile([128, 2, HD], BF16, tag="wk")
        wv_sb = consts.tile([128, 2, HD], BF16, tag="wv")
        wo_r = consts.tile([128, 4, DIM], F32R, tag="wor")
        idbf = consts.tile([128, 128], BF16, tag="idbf")
        ones_f = consts.tile([128, 128], F32, tag="ones_f")
        ones_bf = consts.tile([128, 128], BF16, tag="ones_bf")
        for wext, wsb in ((wq_ext, wq_sb), (wk_ext, wk_sb), (wv_ext, wv_sb)):
            wf = wstage.tile([128, 2, HD], F32, tag="wf")
            nc.scalar.dma_start(
                out=wf[:], in_=wext[:, :].rearrange("(k p) n -> p k n", p=128)
            )
            nc.any.tensor_copy(wsb[:], wf[:])
        wof = wstage.tile([128, 4, DIM], F32, tag="wf")
        nc.scalar.dma_start(
            out=wof[:], in_=wo_ext[:, :].rearrange("(k p) n -> p k n", p=128)
        )
        nc.any.tensor_copy(wo_r[:], wof[:])
        make_identity(nc, idbf[:])
        nc.vector.memset(ones_f[:], 1.0)
        nc.vector.tensor_copy(ones_bf[:], ones_f[:])

        # warm up ncfw so the first real AllReduce skips the cold-start lag
        nc.sync.dma_start(out=wu_in[:, :], in_=idbf[:, 0:8])
        nc.gpsimd.collective_compute(
            "AllReduce",
            mybir.AluOpType.add,
            replica_groups=RG,
            ins=[wu_in[:, :].opt()],
            outs=[wu_out[:, :].opt()],
        )

        attn = attnT_pool.tile([128, H, 4, N], BF16, tag="attn")
        xT = xT_pool.tile([128, 2, RN], BF16, tag="xT")

        proj_psum = tc.alloc_tile_pool(name="proj_psum", bufs=3, space="PSUM")
        dots_psum = tc.alloc_tile_pool(name="dots_psum", bufs=3, space="PSUM")
        xp_psum = tc.alloc_tile_pool(name="xp_psum", bufs=2, space="PSUM")

        # ---- load x, cast to bf16, transpose to x^T [dim(2x128), rn] ----
        # 4 PE transposes batched per PSUM bank -> one [128,512] copy out
        _cast = [0]
        for c4 in range(RN // N):
            xbs = []
            for j in range(4):
                c = c4 * 4 + j
                if c < 4:
                    xr = first_xrs[c]
                else:
                    xr = xrow_pool.tile([128, DIM], F32, tag="xr")
                    # the x load paces the whole kernel front and is
                    # queue-serialized; put 3 of every 8 chunks on the Scalar
                    # queue (idle early) so two DMAs run concurrently
                    if c % 8 >= 5:
                        nc.scalar.dma_start(out=xr[:], in_=x_ext[c * 128:(c + 1) * 128, :])
                    else:
                        nc.sync.dma_start(out=xr[:], in_=x_ext[c * 128:(c + 1) * 128, :])
                xb = xbf_pool.tile([128, DIM], BF16, tag="xb")
                # fp32->bf16 casts: round-robin DVE / ScalarE (GpSimd would
                # sit behind the warmup collective on its queue)
                if _cast[0] % 2 == 0:
                    nc.vector.tensor_copy(xb[:], xr[:])
                else:
                    nc.scalar.copy(xb[:], xr[:])
                _cast[0] += 1
                xbs.append(xb)
            for kc in range(2):
                pt = xp_psum.tile([128, N], BF16, tag="xp")
                for j in range(4):
                    nc.tensor.transpose(
                        pt[:, j * 128:(j + 1) * 128],
                        xbs[j][:, kc * 128:(kc + 1) * 128],
                        idbf[:],
                    )
                cp(xT[:, kc, c4 * N:(c4 + 1) * N], pt[:])

        wstage.release()
        xbf_pool.release()
        xrow_pool.release()
        xp_psum.release()

        dstage_pool = tc.alloc_tile_pool(name="dstage", bufs=4)
        smax_pool = tc.alloc_tile_pool(name="smax", bufs=2)
        qkT_pool = tc.alloc_tile_pool(name="qkT", bufs=1)

        def softmax_local(hp, spool, zpool, sfx, wait_ms):
            """exp + transpose-free normalize of both heads of AllReduce #hp
            into attn[:, 2hp+m, :, :] (tiles are [j-part, i-free]).

            wait_ms biases the Tile scheduler: this whole chain is gated on
            AllReduce #hp, so model it as not-ready before then — otherwise
            its ops head-block the Sync/Scalar/Vector queues ahead of
            independent work (the scheduler's collective timing is a guess)."""
            with tc.tile_wait_until(wait_ms):
                for m in range(2):
                    h = 2 * hp + m
                    exps = []
                    for jc in range(4):
                        zt = spool.tile([128, N], BF16, tag="zt" + sfx)
                        row0 = m * N + jc * 128
                        dma(zt[:], ar_out[hp][row0:row0 + 128, :])
                        et = spool.tile([128, N], BF16, tag="et" + sfx, bufs=5)
                        nc.scalar.activation(
                            et[:], zt[:], mybir.ActivationFunctionType.Exp, scale=scale
                        )
                        exps.append(et)
                    # Z[i] broadcast to all partitions: all-ones stationary
                    # sums exp over the partition (j) axis into every row
                    bps = zpool.tile([128, N], F32, tag="bps" + sfx)
                    for jc in range(4):
                        nc.tensor.matmul(
                            bps[:],
                            ones_bf[:],
                            exps[jc][:],
                            start=(jc == 0),
                            stop=(jc == 3),
                        )
                    rz = spool.tile([128, N], BF16, tag="rz" + sfx)
                    with nc.allow_low_precision(reason="1/Z scale fine in bf16"):
                        nc.vector.reciprocal(rz[:], bps[:])
                    for jc in range(4):
                        nc.vector.tensor_mul(attn[:, h, jc, :], exps[jc][:], rz[:])

        for hp in range(4):
            qT = qkT_pool.tile([128, RN], BF16, tag="qT")
            kT = qkT_pool.tile([128, RN], BF16, tag="kT")
            for wsb, dstT in ((wq_sb, qT), (wk_sb, kT)):
                for ch in range(RN // N):
                    ps = proj_psum.tile([128, N], F32, tag="proj")
                    for kc in range(2):
                        nc.tensor.matmul(
                            ps[:],
                            wsb[:, kc, hp * 128:(hp + 1) * 128],
                            xT[:, kc, ch * N:(ch + 1) * N],
                            start=(kc == 0),
                            stop=(kc == 1),
                        )
                    cp(dstT[:, ch * N:(ch + 1) * N], ps[:])

            # partial dotsT[j,i] for the two heads of this pair; the even head
            # uses PE row-group 0-63, the odd head 64-127 (concurrent tiles)
            for jc in range(4):
                pe_ = dots_psum.tile([128, N], F32, tag="dots")
                po_ = dots_psum.tile([128, N], F32, tag="dots")
                for rr in range(R):
                    base = rr * N
                    jsl = slice(base + jc * 128, base + jc * 128 + 128)
                    isl = slice(base, base + N)
                    nc.tensor.matmul(
                        pe_[:],
                        kT[0:64, jsl],
                        qT[0:64, isl],
                        start=(rr == 0),
                        stop=(rr == R - 1),
                        skip_group_check=True,
                    )
                    nc.tensor.matmul(
                        po_[:],
                        kT[64:128, jsl],
                        qT[64:128, isl],
                        start=(rr == 0),
                        stop=(rr == R - 1),
                        skip_group_check=True,
                    )
                for m, ps in ((0, pe_), (1, po_)):
                    st = dstage_pool.tile([128, N], BF16, tag="dstage")
                    cp(st[:], ps[:])
                    row0 = m * N + jc * 128
                    dma(ar_in[hp][row0:row0 + 128, :], st[:])

            nc.gpsimd.collective_compute(
                "AllReduce",
                mybir.AluOpType.add,
                replica_groups=RG,
                ins=[ar_in[hp][:, :].opt()],
                outs=[ar_out[hp][:, :].opt()],
            )

        qkT_pool.release()
        z_psum = tc.alloc_tile_pool(name="z_psum", bufs=2, space="PSUM")

        # ---- v projection (overlaps the AllReduces; reads xT) ----
        v_tiles = {}
        for rr in range(R):
            if rr == 3:
                softmax_local(0, smax_pool, z_psum, "", 0.17)
            if rr == 9:
                softmax_local(1, smax_pool, z_psum, "", 0.24)
            for jt in range(4):
                ps = proj_psum.tile([128, N], F32, tag="proj")
                for kc in range(2):
                    nc.tensor.matmul(
                        ps[:],
                        xT[:, kc, rr * N + jt * 128:rr * N + jt * 128 + 128],
                        wv_sb[:, kc, :],
                        start=(kc == 0),
                        stop=(kc == 1),
                    )
                vt = v_pool.tile([128, HD], BF16, tag="v")
                cp(vt[:], ps[:])
                v_tiles[(rr, jt)] = vt

        z_psum.release()
        smax_pool.release()
        dstage_pool.release()
        xT_pool.release()
        dots_psum.release()
        proj_psum.release()

        # ---- attn^T @ v -> out^T (f32r), then out @ Wo ----
        # r processed in quarters: all four head-pair blocks for 4 rows, then
        # their output projection; softmax of the last pairs lands between the
        # first quarter's hp2 and hp3 blocks so their AllReduces stay hidden
        oT_pool = tc.alloc_tile_pool(name="oT", bufs=16)
        fstage_pool = tc.alloc_tile_pool(name="fstage", bufs=6)
        av_psum = tc.alloc_tile_pool(name="av_psum", bufs=3, space="PSUM")
        fin_psum = tc.alloc_tile_pool(name="fin_psum", bufs=3, space="PSUM")
        z2_psum = tc.alloc_tile_pool(name="z2_psum", bufs=2, space="PSUM")

        _oq = [0]
        for rq in range(4):
            oTs = {}
            for hp in range(4):
                if rq == 0 and hp == 2:
                    softmax_local(2, fstage_pool, z2_psum, "2", 0.31)
                if rq == 0 and hp == 3:
                    softmax_local(3, fstage_pool, z2_psum, "2", 0.38)
                for rx in range(4):
                    rr = rq * 4 + rx
                    ps = av_psum.tile([128, N], F32, tag="av")
                    for jt in range(4):
                        for m in range(2):
                            h = 2 * hp + m
                            nc.tensor.matmul(
                                ps[m * 64:(m + 1) * 64, :],
                                v_tiles[(rr, jt)][:, h * D:(h + 1) * D],
                                attn[:, h, jt, :],
                                start=(jt == 0),
                                stop=(jt == 3),
                                tile_position=(0, m * 64),
                                skip_group_check=True,
                            )
                    oT = oT_pool.tile([128, N], F32R, tag="oT")
                    cp(oT[:], ps[:])
                    oTs[(rx, hp)] = oT
            for rx in range(4):
                rr = rq * 4 + rx
                for ic in range(4):
                    psf = fin_psum.tile([128, DIM], F32, tag="fin")
                    for kc in range(4):
                        nc.tensor.matmul(
                            psf[:],
                            oTs[(rx, kc)][:, ic * 128:(ic + 1) * 128],
                            wo_r[:, kc, :],
                            start=(kc == 0),
                            stop=(kc == 3),
                        )
                    fst = fstage_pool.tile([128, DIM], F32, tag="fst")
                    cp(fst[:], psf[:])
                    row0 = rr * N + ic * 128
                    # rq>=2 stores run well past the last collective: split
                    # them onto the (now idle) GpSimd queue as a second lane
                    if rq >= 2 and _oq[0] % 2 == 0:
                        nc.gpsimd.dma_start(out=out_ext[row0:row0 + 128, :], in_=fst[:])
                    else:
                        nc.sync.dma_start(out=out_ext[row0:row0 + 128, :], in_=fst[:])
                    _oq[0] += 1

        z2_psum.release()
        fin_psum.release()
        av_psum.release()
        fstage_pool.release()
        oT_pool.release()
        attnT_pool.release()
        v_pool.release()
        consts.release()
        dram.release()

    if not nc.is_finalized():
        nc.finalize()
    return nc


_cache = {}


def _get_nc(scale: float):
    key = round(float(scale), 12)
    if key not in _cache:
        _cache[key] = build_nc(float(scale))
    return _cache[key]


def make_in_maps(x, Wq, Wkv, Wo):
    x = np.ascontiguousarray(np.asarray(x, dtype=np.float32)).reshape(CORES, RN, DIM)
    Wq = np.ascontiguousarray(np.asarray(Wq, dtype=np.float32))
    Wkv = np.asarray(Wkv, dtype=np.float32)
    Wk = np.ascontiguousarray(Wkv[:, :HD])
    Wv = np.ascontiguousarray(Wkv[:, HD:])
    Wo = np.ascontiguousarray(np.asarray(Wo, dtype=np.float32))
    return [
        {"x": x[c], "wq": Wq, "wk": Wk, "wv": Wv, "wo": Wo} for c in range(CORES)
    ]


def kernel(x, Wq, Wkv, Wo, bo, mask, tie_attn_dim):
    x = np.asarray(x)
    br, n, dim = x.shape
    r = int(tie_attn_dim)
    assert (br, n, dim) == (128, 512, 256) and r == 128, "kernel hardcodes shapes"
    mask = np.asarray(mask)
    assert mask.all(), "kernel assumes an all-valid mask"
    num_rows = float(mask.reshape(1, r, n).any(axis=-1).sum(axis=-1)[0])
    scale = (D ** -0.5) * (num_rows ** -0.5)

    nc = _get_nc(scale)
    in_maps = make_in_maps(x, Wq, Wkv, Wo)
    res = bass_utils.run_bass_kernel_spmd(nc, in_maps, core_ids=list(range(CORES)))
    out = np.concatenate([m["out"] for m in res.results], axis=0)
    out = out.reshape(br, n, dim)
    bo = np.asarray(bo, dtype=np.float32)
    if bo.any():
        out = out + bo
    return np.ascontiguousarray(out.astype(np.float32))
